# revision 11
# baseline (speedup 1.0000x reference)
"""Trainium2 Bass kernel for nn_ConvexReLU.

Math: out[i,m] = sum_{j,k,l} G[j,k] * x[i,k,l] * (v-w)[j,l,m]

Reassociated as:
    d = v - w                              (host, elementwise)
    T[k,l,m]   = sum_j G[j,k] * d[j,l,m]   (device matmul, 68.7 GFLOP)
    out[i,m]   = sum_{k,l} x[i,k,l] * T[k,l,m]   (device matmul, 17.2 GFLOP)

Sharding: split l (in_dim, 256) across 8 cores (32 each). Each core computes
a full-shape (out_dim, batch) partial; host sums the 8 partials.

Device layout per core:
    g  : (1024 j, 1024 k)        full G, replicated; 2KB/partition descriptors
    d  : (4 pair, 8 jc, 128 p, 1024) l-shard of v-w, pg-pair-major so each
         DMA descriptor is 2KB contiguous; fully prefetched (bufs=4)
    xt : (32 l, 128 p, 8 kt, 256 i) l-shard of x, partition-major so each
         descriptor is 4KB contiguous
    out: (128 m, 256 i)          partial of out^T

Head optimizations vs v1: tiny first DMA bites (g[:,0:128] + d[:,0:256]) with
column-split first matmuls so the PE starts as soon as ~96KB lands; dummy
warmup matmuls on a memset tile burn the PE pstate ramp during the DMA-feed
latency window; d fully prefetched so stage-1 never waits on d past pg0.
Tail: final PSUM->SBUF copy + DRAM DMA split in halves across both rings.
"""

import os
import sys

import numpy as np

for _p in ("/opt/trn_rl_repo", "/root/.axon_site/_ro/trn_rl_repo"):
    if os.path.isdir(_p) and _p not in sys.path:
        sys.path.insert(0, _p)

import concourse.bass as bass
import concourse.bacc as bacc
import concourse.mybir as mybir
from concourse.bass_utils import run_bass_kernel_spmd
from concourse.tile import TileContext

B, J, K, L, M = 256, 1024, 1024, 256, 128
NCORES = 8
LC = L // NCORES          # 32 l-values per core
NPG = 8                   # l-groups per core
LG = LC // NPG            # 4 l-values per group
NKT = K // 128            # 8 k-tiles
NJC = J // 128            # 8 j-chunks
NPAIR = NPG // 2          # pg-pairs for d tiles

F32 = mybir.dt.float32
F32R = mybir.dt.float32r
BF16 = mybir.dt.bfloat16

DTYPE = os.environ.get("BASS_KERNEL_DTYPE", "bf16")
N_WARM = int(os.environ.get("BASS_N_WARM", "88"))


def _dtypes(dtype_name: str):
    if dtype_name == "bf16":
        return BF16, BF16
    if dtype_name == "mixed":
        return F32R, BF16
    return F32R, F32R


def build_nc(dtype_name: str = DTYPE) -> bass.Bass:
    gd_dt, s2_dt = _dtypes(dtype_name)

    nc = bacc.Bacc(None, debug=False)

    g = nc.declare_dram_parameter("g", [J, K], gd_dt, isOutput=False)
    # d: (pair, jc, p, pair_cols) so each partition row is 2KB contiguous
    d = nc.declare_dram_parameter(
        "d", [NPAIR, NJC, 128, 2 * LG * M], gd_dt, isOutput=False
    )
    # xt: (l, p, kt*i) so each partition row is 4KB contiguous
    xt = nc.declare_dram_parameter("xt", [LC, 128, NKT * B], s2_dt, isOutput=False)
    out = nc.declare_dram_parameter("out", [M, B], F32, isOutput=True)

    g_r = g.rearrange("(jc p) k -> p jc k", p=128)
    d_r = d.rearrange("t jc p f -> t p jc f")
    xt_r = xt.rearrange("l p (kt i) -> l p kt i", kt=NKT)
    PW = 2 * LG * M  # 1024: columns per pg-pair in a d tile

    with TileContext(nc) as tc:
        with (
            tc.tile_pool(name="wpool", bufs=1) as wpool,
            tc.tile_pool(name="gpool", bufs=1) as gpool,
            tc.tile_pool(name="dpool", bufs=4) as dpool,
            tc.tile_pool(name="tpool", bufs=3) as tpool,
            tc.tile_pool(name="xpool", bufs=8) as xpool,
            tc.tile_pool(name="opool", bufs=1) as opool,
            tc.tile_pool(name="ps1", bufs=6, space="PSUM") as ps1,
            tc.tile_pool(name="pso", bufs=1, space="PSUM") as pso,
            tc.tile_pool(name="psw", bufs=1, space="PSUM") as psw,
        ):
            # ---- PE warmup: burn the DVFS pstate ramp on dummy matmuls
            # while the first real operands are still in DMA flight.
            warm = wpool.tile([128, 32], gd_dt)
            nc.gpsimd.memset(warm[:], 0)
            warm_ps = psw.tile([32, 32], F32)
            for _ in range(N_WARM):
                nc.tensor.matmul(
                    warm_ps[:], warm[:], warm[:],
                    start=True, stop=True, skip_group_check=True,
                )

            # ---- head DMAs: alternate rings per jc chunk; first bites tiny
            # so the first matmul's operands land as early as possible.
            g_sb = gpool.tile([128, NJC, K], gd_dt)
            d_tiles = []
            d_sb0 = dpool.tile([128, NJC, PW], gd_dt, tag="d", name="d_p0")
            d_tiles.append(d_sb0)

            # first bites: just enough for the first few matmuls (jc0)
            nc.sync.dma_start(out=g_sb[:, 0, 0:256], in_=g_r[:, 0, 0:256])
            nc.scalar.dma_start(out=d_sb0[:, 0, 0:512], in_=d_r[0, :, 0, 0:512])
            nc.sync.dma_start(out=g_sb[:, 0, 256:1024], in_=g_r[:, 0, 256:1024])
            nc.scalar.dma_start(out=d_sb0[:, 0, 512:1024], in_=d_r[0, :, 0, 512:1024])
            for jc in range(1, NJC):
                ga = nc.sync if jc % 2 == 0 else nc.scalar
                da = nc.scalar if jc % 2 == 0 else nc.sync
                ga.dma_start(out=g_sb[:, jc, :], in_=g_r[:, jc, :])
                da.dma_start(out=d_sb0[:, jc, :], in_=d_r[0, :, jc, :])
            # d pairs 1..3 get one coalesced DMA each (2KB descriptors) but
            # are issued inside the pg loop, after the preceding stage-2's x
            # DMAs, so they don't clog the ring feed ahead of x.
            for t in range(1, NPAIR):
                d_tiles.append(
                    dpool.tile([128, NJC, PW], gd_dt, tag="d", name=f"d_p{t}")
                )

            out_ps = pso.tile([M, B], F32)

            total_mm2 = NPG * LG * NKT
            # kt-groups per stage-1 pass: (6,2) so each jc chunk yields 6
            # back-to-back matmuls early on; psum: 6 stage-1 + 1 out + 1 warm
            KGROUPS = [(0, 6), (6, 2)]
            KH = 4  # stage-2 kt-group width

            mm2_state = [0]

            def stage2(pg, t_sb):
                # out^T += T^T-slices @ x^T-slices for l-group pg.
                xs = []
                for dl in range(LG):
                    x_sb = xpool.tile(
                        [128, NKT, B], s2_dt, tag="x", name=f"x_{pg}_{dl}"
                    )
                    ring = nc.sync if dl % 2 == 0 else nc.scalar
                    ring.dma_start(out=x_sb[:], in_=xt_r[pg * LG + dl])
                    xs.append(x_sb)
                for half in range(NKT // KH):
                    for dl in range(LG):
                        for kt2 in range(KH):
                            kt = half * KH + kt2
                            nc.tensor.matmul(
                                out_ps[:],
                                t_sb[:, kt, dl * M : (dl + 1) * M],
                                xs[dl][:, kt, :],
                                start=(mm2_state[0] == 0),
                                stop=(mm2_state[0] == total_mm2 - 1),
                                skip_group_check=True,
                            )
                            mm2_state[0] += 1

            prev = None  # (pg, t_sb) whose stage-2 is pending

            for pg in range(NPG):
                d_sb = d_tiles[pg // 2]
                dc0 = (pg % 2) * LG * M      # column offset of this pg in pair
                dc1 = dc0 + LG * M

                t_sb = tpool.tile([128, NKT, LG * M], s2_dt, tag="t")
                for gi, (k0, kn) in enumerate(KGROUPS):
                    p1s = [
                        ps1.tile([128, LG * M], F32, tag="p1",
                                 name=f"p1_{pg}_{gi}_{i}")
                        for i in range(kn)
                    ]
                    for jc in range(NJC):
                        for kt2 in range(kn):
                            kt = k0 + kt2
                            nc.tensor.matmul(
                                p1s[kt2][:],
                                g_sb[:, jc, kt * 128 : (kt + 1) * 128],
                                d_sb[:, jc, dc0:dc1],
                                start=(jc == 0),
                                stop=(jc == NJC - 1),
                                skip_group_check=True,
                            )
                    for kt2 in range(kn):
                        kt = k0 + kt2
                        nc.vector.tensor_copy(out=t_sb[:, kt, :], in_=p1s[kt2][:])

                # stage-2 lags stage-1 by one l-group
                if prev is not None:
                    stage2(*prev)
                prev = (pg, t_sb)
                # d pair t is needed by pg=2t; issue its DMA behind the x
                # DMAs of stage2(pg-1) so x isn't starved on the rings
                if pg % 2 == 1 and pg // 2 + 1 < NPAIR:
                    t = pg // 2 + 1
                    ring = nc.sync if t % 2 == 1 else nc.scalar
                    ring.dma_start(out=d_tiles[t][:], in_=d_r[t])

            stage2(*prev)

            out_sb = opool.tile([M, B], F32)
            nc.vector.tensor_copy(out=out_sb[:, 0:128], in_=out_ps[:, 0:128])
            nc.sync.dma_start(out=out[:, 0:128], in_=out_sb[:, 0:128])
            nc.vector.tensor_copy(out=out_sb[:, 128:256], in_=out_ps[:, 128:256])
            nc.scalar.dma_start(out=out[:, 128:256], in_=out_sb[:, 128:256])

    nc.finalize()
    return nc


_NC_CACHE: dict[str, bass.Bass] = {}


def _get_nc(dtype_name: str = DTYPE) -> bass.Bass:
    if dtype_name not in _NC_CACHE:
        _NC_CACHE[dtype_name] = build_nc(dtype_name)
    return _NC_CACHE[dtype_name]


def make_in_maps(x, G, v, w, dtype_name: str = DTYPE):
    x = np.asarray(x, dtype=np.float32)
    G = np.asarray(G, dtype=np.float32)
    v = np.asarray(v, dtype=np.float32)
    w = np.asarray(w, dtype=np.float32)

    d_full = v - w  # (J, L, M)

    import ml_dtypes

    if dtype_name == "bf16":
        gd_np, x_np = ml_dtypes.bfloat16, ml_dtypes.bfloat16
    elif dtype_name == "mixed":
        gd_np, x_np = np.float32, ml_dtypes.bfloat16
    else:
        gd_np, x_np = np.float32, np.float32

    G_io = np.ascontiguousarray(G.astype(gd_np))
    in_maps = []
    for c in range(NCORES):
        ls = slice(c * LC, (c + 1) * LC)
        # d (J, LC, M) -> (pair, jc, p, 2*LG*M): pair-major, 2KB rows
        d_c = d_full[:, ls, :].reshape(NJC, 128, NPAIR, 2 * LG, M)
        d_c = np.ascontiguousarray(
            d_c.transpose(2, 0, 1, 3, 4).reshape(NPAIR, NJC, 128, 2 * LG * M)
            .astype(gd_np)
        )
        # x (B, K, L) -> xt (LC, p, kt*i): partition-major, 4KB rows
        xt_c = x[:, :, ls].transpose(2, 1, 0).reshape(LC, NKT, 128, B)
        xt_c = np.ascontiguousarray(
            xt_c.transpose(0, 2, 1, 3).reshape(LC, 128, NKT * B).astype(x_np)
        )
        in_maps.append({"g": G_io, "d": d_c, "xt": xt_c})
    return in_maps


def kernel(x, G, v, w):
    nc = _get_nc()
    in_maps = make_in_maps(x, G, v, w)
    res = run_bass_kernel_spmd(nc, in_maps, core_ids=list(range(NCORES)))
    acc = np.zeros((M, B), dtype=np.float64)
    for r in res.results:
        acc += r["out"].astype(np.float64)
    return np.ascontiguousarray(acc.T.astype(np.float32))


# revision 14
# speedup vs baseline: 1.0104x; 1.0104x over previous
"""Trainium2 Bass kernel for nn_ConvexReLU.

Math: out[i,m] = sum_{j,k,l} G[j,k] * x[i,k,l] * (v-w)[j,l,m]

Reassociated as:
    d = v - w                              (host, elementwise)
    T[k,l,m]   = sum_j G[j,k] * d[j,l,m]   (device matmul, 68.7 GFLOP)
    out[i,m]   = sum_{k,l} x[i,k,l] * T[k,l,m]   (device matmul, 17.2 GFLOP)

Sharding: split l (in_dim, 256) across 8 cores (32 each). Each core computes
a full-shape (out_dim, batch) partial; host sums the 8 partials.

Device layout per core:
    g  : (1024 j, 1024 k)        full G, replicated; 2KB/partition descriptors
    d  : (4 pair, 8 jc, 128 p, 1024) l-shard of v-w, pg-pair-major so each
         DMA descriptor is 2KB contiguous; fully prefetched (bufs=4)
    xt : (32 l, 128 p, 8 kt, 256 i) l-shard of x, partition-major so each
         descriptor is 4KB contiguous
    out: (128 m, 256 i)          partial of out^T

Head optimizations vs v1: tiny first DMA bites (g[:,0:128] + d[:,0:256]) with
column-split first matmuls so the PE starts as soon as ~96KB lands; dummy
warmup matmuls on a memset tile burn the PE pstate ramp during the DMA-feed
latency window; d fully prefetched so stage-1 never waits on d past pg0.
Tail: final PSUM->SBUF copy + DRAM DMA split in halves across both rings.
"""

import os
import sys

import numpy as np

for _p in ("/opt/trn_rl_repo", "/root/.axon_site/_ro/trn_rl_repo"):
    if os.path.isdir(_p) and _p not in sys.path:
        sys.path.insert(0, _p)

import concourse.bass as bass
import concourse.bacc as bacc
import concourse.mybir as mybir
from concourse.bass_utils import run_bass_kernel_spmd
from concourse.tile import TileContext

B, J, K, L, M = 256, 1024, 1024, 256, 128
NCORES = 8
LC = L // NCORES          # 32 l-values per core
NPG = 8                   # l-groups per core
LG = LC // NPG            # 4 l-values per group
NKT = K // 128            # 8 k-tiles
NJC = J // 128            # 8 j-chunks
NPAIR = NPG // 2          # pg-pairs for d tiles

F32 = mybir.dt.float32
F32R = mybir.dt.float32r
BF16 = mybir.dt.bfloat16

DTYPE = os.environ.get("BASS_KERNEL_DTYPE", "bf16")
N_WARM = int(os.environ.get("BASS_N_WARM", "0"))
G_OFFLOAD = int(os.environ.get("BASS_G_OFFLOAD", "0"))


def _dtypes(dtype_name: str):
    if dtype_name == "bf16":
        return BF16, BF16
    if dtype_name == "mixed":
        return F32R, BF16
    return F32R, F32R


def build_nc(dtype_name: str = DTYPE) -> bass.Bass:
    gd_dt, s2_dt = _dtypes(dtype_name)

    nc = bacc.Bacc(None, debug=False)

    g = nc.declare_dram_parameter("g", [J, K], gd_dt, isOutput=False)
    # d: (pair, jc, p, pair_cols) so each partition row is 2KB contiguous
    d = nc.declare_dram_parameter(
        "d", [NPAIR, NJC, 128, 2 * LG * M], gd_dt, isOutput=False
    )
    # xt: (l, p, kt*i) so each partition row is 4KB contiguous
    xt = nc.declare_dram_parameter("xt", [LC, 128, NKT * B], s2_dt, isOutput=False)
    out = nc.declare_dram_parameter("out", [M, B], F32, isOutput=True)

    g_r = g.rearrange("(jc p) k -> p jc k", p=128)
    d_r = d.rearrange("t jc p f -> t p jc f")
    xt_r = xt.rearrange("l p (kt i) -> l p kt i", kt=NKT)
    PW = 2 * LG * M  # 1024: columns per pg-pair in a d tile

    with TileContext(nc) as tc:
        with (
            tc.tile_pool(name="wpool", bufs=1) as wpool,
            tc.tile_pool(name="gpool", bufs=1) as gpool,
            tc.tile_pool(name="dpool", bufs=4) as dpool,
            tc.tile_pool(name="tpool", bufs=3) as tpool,
            tc.tile_pool(name="xpool", bufs=8) as xpool,
            tc.tile_pool(name="opool", bufs=1) as opool,
            tc.tile_pool(name="ps1", bufs=6, space="PSUM") as ps1,
            tc.tile_pool(name="pso", bufs=1, space="PSUM") as pso,
            tc.tile_pool(name="psw", bufs=1, space="PSUM") as psw,
        ):
            # ---- optional PE warmup: burn the DVFS pstate ramp on dummy
            # matmuls while the first real operands are in DMA flight.
            if N_WARM:
                warm = wpool.tile([128, 32], gd_dt)
                nc.vector.memset(warm[:], 0)
                warm_ps = psw.tile([32, 32], F32)
                for _ in range(N_WARM):
                    nc.tensor.matmul(
                        warm_ps[:], warm[:], warm[:],
                        start=True, stop=True, skip_group_check=True,
                    )

            # ---- head DMAs: alternate rings per jc chunk; first bites tiny
            # so the first matmul's operands land as early as possible.
            g_sb = gpool.tile([128, NJC, K], gd_dt)
            d_tiles = []
            d_sb0 = dpool.tile([128, NJC, PW], gd_dt, tag="d", name="d_p0")
            d_tiles.append(d_sb0)

            # head: pg0 columns only (baseline supply profile); the first g
            # piece is split so kt0's weights land first. g chunks jc>=4 can
            # optionally be offloaded to the gpsimd software-DGE ring to
            # relieve the two HWDGE rings during the head crunch.
            nc.sync.dma_start(out=g_sb[:, 0, 0:256], in_=g_r[:, 0, 0:256])
            nc.scalar.dma_start(out=d_sb0[:, 0, 0:512], in_=d_r[0, :, 0, 0:512])
            nc.sync.dma_start(out=g_sb[:, 0, 256:1024], in_=g_r[:, 0, 256:1024])
            if G_OFFLOAD:
                nc.gpsimd.dma_start(
                    out=g_sb[:, 4:NJC, :], in_=g_r[:, 4:NJC, :]
                )
            for jc in range(1, NJC):
                ga = nc.sync if jc % 2 == 0 else nc.scalar
                da = nc.scalar if jc % 2 == 0 else nc.sync
                if not (G_OFFLOAD and jc >= 4):
                    ga.dma_start(out=g_sb[:, jc, :], in_=g_r[:, jc, :])
                da.dma_start(
                    out=d_sb0[:, jc, 0:512], in_=d_r[0, :, jc, 0:512]
                )
            # pg1 columns of pair0: one coalesced DMA, needed only at ~pg1
            nc.scalar.dma_start(
                out=d_sb0[:, :, 512:1024], in_=d_r[0, :, :, 512:1024]
            )
            # d pairs 1..3 get one coalesced DMA each (2KB descriptors) but
            # are issued inside the pg loop, after the preceding stage-2's x
            # DMAs, so they don't clog the ring feed ahead of x.
            for t in range(1, NPAIR):
                d_tiles.append(
                    dpool.tile([128, NJC, PW], gd_dt, tag="d", name=f"d_p{t}")
                )

            out_ps = pso.tile([M, B], F32)

            total_mm2 = NPG * LG * NKT
            # kt-groups per stage-1 pass: (6,2) so each jc chunk yields 6
            # back-to-back matmuls early on; psum: 6 stage-1 + 1 out + 1 warm
            KGROUPS = [(0, 6), (6, 2)]
            KH = 4  # stage-2 kt-group width

            mm2_state = [0]

            def stage2(pg, t_sb):
                # out^T += T^T-slices @ x^T-slices for l-group pg.
                xs = []
                for dl in range(LG):
                    x_sb = xpool.tile(
                        [128, NKT, B], s2_dt, tag="x", name=f"x_{pg}_{dl}"
                    )
                    ring = nc.sync if dl % 2 == 0 else nc.scalar
                    ring.dma_start(out=x_sb[:], in_=xt_r[pg * LG + dl])
                    xs.append(x_sb)
                for half in range(NKT // KH):
                    for dl in range(LG):
                        for kt2 in range(KH):
                            kt = half * KH + kt2
                            nc.tensor.matmul(
                                out_ps[:],
                                t_sb[:, kt, dl * M : (dl + 1) * M],
                                xs[dl][:, kt, :],
                                start=(mm2_state[0] == 0),
                                stop=(mm2_state[0] == total_mm2 - 1),
                                skip_group_check=True,
                            )
                            mm2_state[0] += 1

            prev = None  # (pg, t_sb) whose stage-2 is pending

            for pg in range(NPG):
                d_sb = d_tiles[pg // 2]
                dc0 = (pg % 2) * LG * M      # column offset of this pg in pair
                dc1 = dc0 + LG * M

                t_sb = tpool.tile([128, NKT, LG * M], s2_dt, tag="t")
                for gi, (k0, kn) in enumerate(KGROUPS):
                    p1s = [
                        ps1.tile([128, LG * M], F32, tag="p1",
                                 name=f"p1_{pg}_{gi}_{i}")
                        for i in range(kn)
                    ]
                    for jc in range(NJC):
                        for kt2 in range(kn):
                            kt = k0 + kt2
                            nc.tensor.matmul(
                                p1s[kt2][:],
                                g_sb[:, jc, kt * 128 : (kt + 1) * 128],
                                d_sb[:, jc, dc0:dc1],
                                start=(jc == 0),
                                stop=(jc == NJC - 1),
                                skip_group_check=True,
                            )
                    for kt2 in range(kn):
                        kt = k0 + kt2
                        nc.vector.tensor_copy(out=t_sb[:, kt, :], in_=p1s[kt2][:])

                # stage-2 lags stage-1 by one l-group
                if prev is not None:
                    stage2(*prev)
                prev = (pg, t_sb)
                # d pair t is needed by pg=2t; issue its DMA behind the x
                # DMAs of stage2(pg-1) so x isn't starved on the rings
                if pg % 2 == 1 and pg // 2 + 1 < NPAIR:
                    t = pg // 2 + 1
                    ring = nc.sync if t % 2 == 1 else nc.scalar
                    ring.dma_start(out=d_tiles[t][:], in_=d_r[t])

            stage2(*prev)

            out_sb = opool.tile([M, B], F32)
            nc.vector.tensor_copy(out=out_sb[:, 0:128], in_=out_ps[:, 0:128])
            nc.sync.dma_start(out=out[:, 0:128], in_=out_sb[:, 0:128])
            nc.vector.tensor_copy(out=out_sb[:, 128:256], in_=out_ps[:, 128:256])
            nc.scalar.dma_start(out=out[:, 128:256], in_=out_sb[:, 128:256])

    nc.finalize()
    return nc


_NC_CACHE: dict[str, bass.Bass] = {}


def _get_nc(dtype_name: str = DTYPE) -> bass.Bass:
    if dtype_name not in _NC_CACHE:
        _NC_CACHE[dtype_name] = build_nc(dtype_name)
    return _NC_CACHE[dtype_name]


def make_in_maps(x, G, v, w, dtype_name: str = DTYPE):
    x = np.asarray(x, dtype=np.float32)
    G = np.asarray(G, dtype=np.float32)
    v = np.asarray(v, dtype=np.float32)
    w = np.asarray(w, dtype=np.float32)

    d_full = v - w  # (J, L, M)

    import ml_dtypes

    if dtype_name == "bf16":
        gd_np, x_np = ml_dtypes.bfloat16, ml_dtypes.bfloat16
    elif dtype_name == "mixed":
        gd_np, x_np = np.float32, ml_dtypes.bfloat16
    else:
        gd_np, x_np = np.float32, np.float32

    G_io = np.ascontiguousarray(G.astype(gd_np))
    in_maps = []
    for c in range(NCORES):
        ls = slice(c * LC, (c + 1) * LC)
        # d (J, LC, M) -> (pair, jc, p, 2*LG*M): pair-major, 2KB rows
        d_c = d_full[:, ls, :].reshape(NJC, 128, NPAIR, 2 * LG, M)
        d_c = np.ascontiguousarray(
            d_c.transpose(2, 0, 1, 3, 4).reshape(NPAIR, NJC, 128, 2 * LG * M)
            .astype(gd_np)
        )
        # x (B, K, L) -> xt (LC, p, kt*i): partition-major, 4KB rows
        xt_c = x[:, :, ls].transpose(2, 1, 0).reshape(LC, NKT, 128, B)
        xt_c = np.ascontiguousarray(
            xt_c.transpose(0, 2, 1, 3).reshape(LC, 128, NKT * B).astype(x_np)
        )
        in_maps.append({"g": G_io, "d": d_c, "xt": xt_c})
    return in_maps


def kernel(x, G, v, w):
    nc = _get_nc()
    in_maps = make_in_maps(x, G, v, w)
    res = run_bass_kernel_spmd(nc, in_maps, core_ids=list(range(NCORES)))
    acc = np.zeros((M, B), dtype=np.float64)
    for r in res.results:
        acc += r["out"].astype(np.float64)
    return np.ascontiguousarray(acc.T.astype(np.float32))


# revision 18
# speedup vs baseline: 1.0267x; 1.0161x over previous
"""Trainium2 Bass kernel for nn_ConvexReLU.

Math: out[i,m] = sum_{j,k,l} G[j,k] * x[i,k,l] * (v-w)[j,l,m]

Reassociated as:
    d = v - w                              (host, elementwise)
    T[k,l,m]   = sum_j G[j,k] * d[j,l,m]   (device matmul, 68.7 GFLOP)
    out[i,m]   = sum_{k,l} x[i,k,l] * T[k,l,m]   (device matmul, 17.2 GFLOP)

Sharding: split l (in_dim, 256) across 8 cores (32 each). Each core computes
a full-shape (out_dim, batch) partial; host sums the 8 partials.

Device layout per core:
    g  : (1024 j, 1024 k)      full G, replicated
    d  : (1024 j, 32 l, 128 m) l-shard of v-w
    xt : (32 l, 1024 k, 256 i) l-shard of x, transposed on host
    out: (128 m, 256 i)        partial of out^T

Default dtype is bf16 (PE multiplies at fp22 internally, accumulates fp32;
measured rel err ~3e-3). BASS_KERNEL_DTYPE=f32r selects a full-precision
variant (~2e-4, ~30% slower). Measured: ~157us/core HW exec at bf16 =
~87% of the 78.6 TF/s per-core tensor-engine roofline for the 10.7
GFLOP/core this decomposition needs.
"""

import os
import sys

import numpy as np

for _p in ("/opt/trn_rl_repo", "/root/.axon_site/_ro/trn_rl_repo"):
    if os.path.isdir(_p) and _p not in sys.path:
        sys.path.insert(0, _p)

import concourse.bass as bass
import concourse.bacc as bacc
import concourse.mybir as mybir
from concourse.bass_utils import run_bass_kernel_spmd
from concourse.tile import TileContext

B, J, K, L, M = 256, 1024, 1024, 256, 128
NCORES = 8
LC = L // NCORES          # 32 l-values per core
NPG = 8                   # l-groups per core
LG = LC // NPG            # 4 l-values per group
NKT = K // 128            # 8 k-tiles
NJC = J // 128            # 8 j-chunks

F32 = mybir.dt.float32
F32R = mybir.dt.float32r
BF16 = mybir.dt.bfloat16

DTYPE = os.environ.get("BASS_KERNEL_DTYPE", "bf16")


def _dtypes(dtype_name: str):
    # (g/d stage-1 dtype, t/x stage-2 dtype). Stage-2 must be dtype-uniform:
    # f32r stationary + bf16 moving takes the explicit-LDWEIGHTS path, which
    # yields all-zero HW output for f32r weights.
    if dtype_name == "bf16":
        return BF16, BF16
    if dtype_name == "mixed":
        return F32R, BF16
    return F32R, F32R


def build_nc(dtype_name: str = DTYPE) -> bass.Bass:
    gd_dt, s2_dt = _dtypes(dtype_name)

    nc = bacc.Bacc(None, debug=False)

    g = nc.declare_dram_parameter("g", [J, K], gd_dt, isOutput=False)
    d = nc.declare_dram_parameter("d", [J, LC, M], gd_dt, isOutput=False)
    # xt partition-major per l so each DMA descriptor is 4KB contiguous
    xt = nc.declare_dram_parameter("xt", [LC, 128, NKT * B], s2_dt, isOutput=False)
    out = nc.declare_dram_parameter("out", [M, B], F32, isOutput=True)

    g_r = g.rearrange("(jc p) k -> p jc k", p=128)
    d_r = d.rearrange("(jc p) l m -> p jc (l m)", p=128)
    xt_r = xt.rearrange("l p (kt i) -> l p kt i", kt=NKT)

    with TileContext(nc) as tc:
        with (
            tc.tile_pool(name="gpool", bufs=1) as gpool,
            tc.tile_pool(name="dpool", bufs=2) as dpool,
            tc.tile_pool(name="tpool", bufs=3) as tpool,
            tc.tile_pool(name="xpool", bufs=8) as xpool,
            tc.tile_pool(name="opool", bufs=1) as opool,
            tc.tile_pool(name="ps1", bufs=6, space="PSUM") as ps1,
            tc.tile_pool(name="pso", bufs=1, space="PSUM") as pso,
        ):
            # per-jc DMAs so the first matmuls unblock after ~0.75 MB, not 6 MB.
            # pg=0's d chunks are interleaved with g chunks: stage-1 consumes
            # (g[jc], d[jc]) pairs in jc order.
            # g on the sync HWDGE ring, d on the scalar HWDGE ring: the two
            # rings dispatch in parallel (~650ns SP issue cost per dma_start).
            g_sb = gpool.tile([128, NJC, K], gd_dt)
            d_sb0 = dpool.tile([128, NJC, LG * M], gd_dt, tag="d")
            for jc in range(NJC):
                ga = nc.sync if jc % 2 == 0 else nc.scalar
                da = nc.scalar if jc % 2 == 0 else nc.sync
                if jc == 0:
                    # split the very first chunk so the first matmul's
                    # operands land ~1us earlier
                    ga.dma_start(out=g_sb[:, 0, 0:256], in_=g_r[:, 0, 0:256])
                    da.dma_start(
                        out=d_sb0[:, 0, :], in_=d_r[:, 0, 0 : LG * M]
                    )
                    ga.dma_start(out=g_sb[:, 0, 256:], in_=g_r[:, 0, 256:])
                else:
                    ga.dma_start(out=g_sb[:, jc, :], in_=g_r[:, jc, :])
                    da.dma_start(
                        out=d_sb0[:, jc, :], in_=d_r[:, jc, 0 : LG * M]
                    )

            out_ps = pso.tile([M, B], F32)

            total_mm2 = NPG * LG * NKT
            # kt-groups per stage-1 pass: (6,2) so each jc chunk yields 6
            # back-to-back matmuls early on (outruns the g/d DMA cadence);
            # psum: 6 live stage-1 banks + 1 out bank <= 8
            KGROUPS = [(0, 6), (6, 2)]
            KH = 4  # stage-2 kt-group width

            mm2_state = [0]

            def stage2(pg, t_sb):
                # out^T += T^T-slices @ x^T-slices for l-group pg.
                # kt-half outer: the first half's matmuls only need the
                # first half of the T copies, overlapping the second half.
                xs = []
                for dl in range(LG):
                    x_sb = xpool.tile(
                        [128, NKT, B], s2_dt, tag="x", name=f"x_{pg}_{dl}"
                    )
                    nc.sync.dma_start(out=x_sb[:], in_=xt_r[pg * LG + dl])
                    xs.append(x_sb)
                for half in range(NKT // KH):
                    for dl in range(LG):
                        for kt2 in range(KH):
                            kt = half * KH + kt2
                            nc.tensor.matmul(
                                out_ps[:],
                                t_sb[:, kt, dl * M : (dl + 1) * M],
                                xs[dl][:, kt, :],
                                start=(mm2_state[0] == 0),
                                stop=(mm2_state[0] == total_mm2 - 1),
                                skip_group_check=True,
                            )
                            mm2_state[0] += 1

            prev = None  # (pg, t_sb) whose stage-2 is pending

            for pg in range(NPG):
                # ---- stage 1: T[k, (l,m)] for this l-group ----
                if pg == 0:
                    d_sb = d_sb0
                else:
                    # steady state: two half-DMAs per pg — coarse enough to
                    # amortize the ~650ns dispatch, fine enough that the
                    # jc-outer loop can start on the first half
                    d_sb = dpool.tile([128, NJC, LG * M], gd_dt, tag="d")
                    nc.scalar.dma_start(
                        out=d_sb[:, 0 : NJC // 2, :],
                        in_=d_r[:, 0 : NJC // 2, pg * LG * M : (pg + 1) * LG * M],
                    )
                    nc.sync.dma_start(
                        out=d_sb[:, NJC // 2 :, :],
                        in_=d_r[:, NJC // 2 :, pg * LG * M : (pg + 1) * LG * M],
                    )

                t_sb = tpool.tile([128, NKT, LG * M], s2_dt, tag="t")
                for gi, (k0, kn) in enumerate(KGROUPS):
                    p1s = [ps1.tile([128, LG * M], F32, tag="p1", name=f"p1_{pg}_{gi}_{i}") for i in range(kn)]
                    # jc-outer: each (g[jc], d[jc]) pair is fully consumed as
                    # soon as its DMA lands -> PE starts ~3us into the kernel
                    for jc in range(NJC):
                        for kt2 in range(kn):
                            kt = k0 + kt2
                            nc.tensor.matmul(
                                p1s[kt2][:],
                                g_sb[:, jc, kt * 128 : (kt + 1) * 128],
                                d_sb[:, jc, :],
                                start=(jc == 0),
                                stop=(jc == NJC - 1),
                                skip_group_check=True,
                            )
                    for kt2 in range(kn):
                        kt = k0 + kt2
                        nc.vector.tensor_copy(out=t_sb[:, kt, :], in_=p1s[kt2][:])

                # stage-2 lags stage-1 by one l-group: during the cold start
                # PE has two stage-1 passes (only g+d needed, ~4 MB) before
                # any x tile is required, hiding the initial DMA crunch.
                if prev is not None:
                    stage2(*prev)
                prev = (pg, t_sb)

            stage2(*prev)

            out_sb = opool.tile([M, B], F32)
            nc.vector.tensor_copy(out=out_sb[:, 0:128], in_=out_ps[:, 0:128])
            nc.sync.dma_start(out=out[:, 0:128], in_=out_sb[:, 0:128])
            nc.vector.tensor_copy(out=out_sb[:, 128:256], in_=out_ps[:, 128:256])
            nc.scalar.dma_start(out=out[:, 128:256], in_=out_sb[:, 128:256])

    nc.finalize()
    return nc


_NC_CACHE: dict[str, bass.Bass] = {}


def _get_nc(dtype_name: str = DTYPE) -> bass.Bass:
    if dtype_name not in _NC_CACHE:
        _NC_CACHE[dtype_name] = build_nc(dtype_name)
    return _NC_CACHE[dtype_name]


def make_in_maps(x, G, v, w, dtype_name: str = DTYPE):
    x = np.asarray(x, dtype=np.float32)
    G = np.asarray(G, dtype=np.float32)
    v = np.asarray(v, dtype=np.float32)
    w = np.asarray(w, dtype=np.float32)

    d_full = v - w  # (J, L, M)

    import ml_dtypes

    if dtype_name == "bf16":
        gd_np, x_np = ml_dtypes.bfloat16, ml_dtypes.bfloat16
    elif dtype_name == "mixed":
        gd_np, x_np = np.float32, ml_dtypes.bfloat16
    else:
        gd_np, x_np = np.float32, np.float32

    G_io = np.ascontiguousarray(G.astype(gd_np))
    in_maps = []
    for c in range(NCORES):
        ls = slice(c * LC, (c + 1) * LC)
        d_c = np.ascontiguousarray(d_full[:, ls, :].astype(gd_np))
        # x (B,K,L) -> xt (LC, K, B)
        # x (B,K,L) -> xt (LC, p, kt*i): partition-major, 4KB contiguous rows
        xt_c = x[:, :, ls].transpose(2, 1, 0).reshape(LC, NKT, 128, B)
        xt_c = np.ascontiguousarray(
            xt_c.transpose(0, 2, 1, 3).reshape(LC, 128, NKT * B).astype(x_np)
        )
        in_maps.append({"g": G_io, "d": d_c, "xt": xt_c})
    return in_maps


def kernel(x, G, v, w):
    nc = _get_nc()
    in_maps = make_in_maps(x, G, v, w)
    res = run_bass_kernel_spmd(nc, in_maps, core_ids=list(range(NCORES)))
    acc = np.zeros((M, B), dtype=np.float64)
    for r in res.results:
        acc += r["out"].astype(np.float64)
    return np.ascontiguousarray(acc.T.astype(np.float32))

